# revision 1
# baseline (speedup 1.0000x reference)
"""Trainium2 Bass kernel for nn_BiDirectionalFusionModule.

Pure batch data-parallelism: 8 samples -> 8 NeuronCores, each core runs the
full module for one sample. All matmuls bf16 with fp32 PSUM accumulation.

Host-side folds: BN affine -> post-conv scale/bias; LN affine + softmax scale
-> K/V 1x1 conv weights; clip(gamma) -> post-attention LN affine. The softmax
denominator and max-subtraction cancel inside the channel-LayerNorm that
follows each attention output, so softmax becomes a bare exp (verified: LN is
invariant to positive per-pixel scaling up to its eps).

3x3 convs = 9 shifted matmuls per 128-channel input tile accumulating in PSUM.
Width handled by pitch-90 rows with zero columns at 0/89; height edges by
row-clipped partial-N matmuls. Cross-partition (channel) reductions for LN
stats via ones-column matmuls; per-pixel stat vectors are reshaped through
DRAM to [121,64] tiles for wide DVE/ACT math, then DMA-broadcast back.
"""
import numpy as np
import ml_dtypes
from contextlib import ExitStack

import concourse.bass as bass
from concourse import bacc
import concourse.tile as tile
import concourse.mybir as mybir
from concourse.bass_utils import run_bass_kernel_spmd

F32 = mybir.dt.float32
BF16 = mybir.dt.bfloat16
AF = mybir.ActivationFunctionType
ALU = mybir.AluOpType
BF = ml_dtypes.bfloat16

B, C, H, W = 8, 256, 88, 88
RR = 8
HR = H // RR                # 11
M2 = HR * HR                # 121
N = H * W                   # 7744
PITCH = 90
EPS = 1e-5
CQ = C // 8                 # 32

BLOCKS = [(i * 5, 5) for i in range(17)] + [(85, 3)]
CHUNK_ROWS = 22             # apply-phase chunking: 4 chunks of 22 rows
PIPE_APPLY = False

(CB_S1, CB_T1, CB_SRB0, CB_SRB1, CB_NG0, CB_NB0, CB_NG1, CB_NB1, CB_FS, CB_FT,
 CB_KB0, CB_QB0, CB_KB1, CB_QB1) = range(14)

_CACHE = {}


def _prep(inputs):
    ii = {k: np.asarray(v, dtype=np.float32) for k, v in inputs.items()}
    scale = float(CQ) ** -0.5

    def fold_bn(g, be, m, v):
        s = g / np.sqrt(v + EPS)
        return s, (0.0 - m) * s + be

    w1T = ii['sm_w1'].transpose(2, 3, 1, 0).reshape(9, 2 * C, C).astype(BF)
    s1, t1 = fold_bn(ii['sm_g1'], ii['sm_be1'], ii['sm_m1'], ii['sm_v1'])
    t1 = t1 + ii['sm_b1'] * s1
    w2T = ii['sm_w2'][:, :, 0, 0].T.astype(BF)
    b2 = float(ii['sm_b2'][0])

    fwT = ii['fus_w'][:, :2 * C].transpose(2, 3, 1, 0).reshape(9, 2 * C, C).astype(BF)
    fwm = ii['fus_w'][:, 2 * C, :, :].transpose(1, 2, 0).reshape(9, C).astype(BF)
    fs, ft = fold_bn(ii['fus_g'], ii['fus_be'], ii['fus_m'], ii['fus_v'])
    ft = ft + ii['fus_b'] * fs

    dirs = {}
    for di, pfx in enumerate(('d2r', 'r2d')):
        g = ii[pfx + '_ln_g']; bl = ii[pfx + '_ln_b']
        kw = ii[pfx + '_k_w'][:, :, 0, 0]; kb = ii[pfx + '_k_b']
        vw = ii[pfx + '_v_w'][:, :, 0, 0]; vb = ii[pfx + '_v_b']
        qw = ii[pfx + '_q_w'][:, :, 0, 0]; qb = ii[pfx + '_q_b']
        gamma = float(np.clip(ii[pfx + '_gamma'], 0.0, 1.0)[0])
        dirs[di] = dict(
            srwT=ii[pfx + '_sr_w'].transpose(2, 3, 1, 0).reshape(64, C, C).astype(BF),
            srb=ii[pfx + '_sr_b'],
            kwT=(scale * kw * g[None, :]).T.astype(BF),
            kb=scale * (kb + kw @ bl),
            qwT=qw.T.astype(BF), qb=qb,
            vwN=(vw * g[None, :]).T.astype(BF),
            vb=(vb + vw @ bl).astype(BF),
            ng=gamma * ii[pfx + '_norm_g'],
            nb=gamma * ii[pfx + '_norm_b'],
        )

    cb = np.zeros((C, 14), np.float32)
    cb[:, CB_S1] = s1; cb[:, CB_T1] = t1
    cb[:, CB_SRB0] = dirs[0]['srb']; cb[:, CB_SRB1] = dirs[1]['srb']
    cb[:, CB_NG0] = dirs[0]['ng']; cb[:, CB_NB0] = dirs[0]['nb']
    cb[:, CB_NG1] = dirs[1]['ng']; cb[:, CB_NB1] = dirs[1]['nb']
    cb[:, CB_FS] = fs; cb[:, CB_FT] = ft
    cb[:CQ, CB_KB0] = dirs[0]['kb']; cb[:CQ, CB_QB0] = dirs[0]['qb']
    cb[:CQ, CB_KB1] = dirs[1]['kb']; cb[:CQ, CB_QB1] = dirs[1]['qb']
    cbp = np.zeros((128, 28), np.float32)
    cbp[:, 0:14] = cb[0:128]; cbp[:, 14:28] = cb[128:256]

    kq = np.zeros((C, 128), BF)
    kq[:, 0:32] = dirs[0]['kwT']; kq[:, 32:64] = dirs[0]['qwT']
    kq[:, 64:96] = dirs[1]['kwT']; kq[:, 96:128] = dirs[1]['qwT']
    vw2 = np.concatenate([dirs[0]['vwN'], dirs[1]['vwN']], axis=1)
    vbr = np.concatenate([dirs[0]['vb'], dirs[1]['vb']])[None, :]

    shared = dict(w1=w1T, w2=w2T, fw=fwT, fwm=fwm, cb=cbp, kq=kq,
                  vw2=np.ascontiguousarray(vw2), vbr=np.ascontiguousarray(vbr),
                  srw0=dirs[0]['srwT'], srw1=dirs[1]['srwT'])

    rgb = ii['f_rgb']; dep = ii['f_depth']
    in_maps = []
    for i in range(B):
        x = np.zeros((2 * C, H, PITCH), BF)
        x[:C, :, 1:89] = rgb[i].astype(BF)
        x[C:, :, 1:89] = dep[i].astype(BF)
        m = dict(shared)
        m['x'] = np.ascontiguousarray(x.reshape(2 * C, H * PITCH))
        in_maps.append(m)
    return in_maps, b2


def _conv3x3(nc, psum, lhsT_of, x_view, y0, nr, n_ci, stop_last, ci_order=None):
    """9*n_ci shifted matmuls accumulating into psum[128, nr*W].
    lhsT_of(off_idx, ci) -> [128,128] AP; x_view(ci, rlo, rhi, dx) -> rhs AP.
    dy==1 offsets first so the initial matmul covers the full region."""
    plan = []
    for dy, dx in [(1, 0), (1, 1), (1, 2), (0, 0), (0, 1), (0, 2),
                   (2, 0), (2, 1), (2, 2)]:
        s = dy - 1
        ylo = max(y0, -s); yhi = min(y0 + nr, H - s)
        if ylo >= yhi:
            continue
        for ci in (ci_order or range(n_ci)):
            plan.append((dy * 3 + dx, ci, s, ylo, yhi))
    for i, (o, ci, s, ylo, yhi) in enumerate(plan):
        out = psum if (ylo == y0 and yhi == y0 + nr) else \
            psum[:, (ylo - y0) * W:(yhi - y0) * W]
        nc.tensor.matmul(out, lhsT_of(o, ci), x_view(ci, ylo + s, yhi + s, o % 3),
                         start=(i == 0), stop=(stop_last and i == len(plan) - 1))


def _build(nc, b2, dbg=False, maxphase=4):
    x_d = nc.dram_tensor("x", [2 * C, H * PITCH], BF16, kind="ExternalInput")
    w1_d = nc.dram_tensor("w1", [9, 2 * C, C], BF16, kind="ExternalInput")
    w2_d = nc.dram_tensor("w2", [C, 1], BF16, kind="ExternalInput")
    fw_d = nc.dram_tensor("fw", [9, 2 * C, C], BF16, kind="ExternalInput")
    fwm_d = nc.dram_tensor("fwm", [9, C], BF16, kind="ExternalInput")
    cb_d = nc.dram_tensor("cb", [128, 28], F32, kind="ExternalInput")
    kq_d = nc.dram_tensor("kq", [C, 128], BF16, kind="ExternalInput")
    vw2_d = nc.dram_tensor("vw2", [C, 2 * C], BF16, kind="ExternalInput")
    vbr_d = nc.dram_tensor("vbr", [1, 2 * C], BF16, kind="ExternalInput")
    srw_d = [nc.dram_tensor("srw0", [64, C, C], BF16, kind="ExternalInput"),
             nc.dram_tensor("srw1", [64, C, C], BF16, kind="ExternalInput")]
    out_d = nc.dram_tensor("out", [C, N], F32, kind="ExternalOutput")
    dbg_d = {}
    if dbg:
        for nm, shp in [("mask", [1, H * PITCH]), ("msk0", [128, N]),
                        ("kvr0", [C, M2]), ("kvr1", [C, M2]),
                        ("kvn0", [C, M2]), ("kvn1", [C, M2]),
                        ("k0", [32, M2]), ("k1", [32, M2]),
                        ("v0", [M2, C]), ("v1", [M2, C]),
                        ("num0", [C, N]), ("num1", [C, N]),
                        ("rm0", [2, N]), ("rm1", [2, N])]:
            dbg_d[nm] = nc.dram_tensor("dbg_" + nm, shp, BF16, kind="ExternalOutput")

    with tile.TileContext(nc) as tc:
        es = ExitStack()
        with es, tc.tile_pool(name="dram", bufs=1, space="DRAM") as dpool:
            gp = es.enter_context(tc.tile_pool(name="gp", bufs=1))

            cb_sb = gp.tile([128, 28], F32, name="cb_sb")

            def cbc(col, half):
                return cb_sb[:, half * 14 + col:half * 14 + col + 1]

            kq_sb = gp.tile([128, 2, 128], BF16, name="kq_sb")
            vw2_sb = gp.tile([128, 2, 2 * C], BF16, name="vw2_sb")
            vbr_sb = gp.tile([1, 2 * C], BF16, name="vbr_sb")
            w2_sb = gp.tile([128, 2, 1], BF16, name="w2_sb")
            ones_bf = gp.tile([128, 1], BF16, name="ones_bf")
            nc.vector.memset(ones_bf, 1.0)
            ones1_bf = gp.tile([1, M2], BF16, name="ones1_bf")
            nc.vector.memset(ones1_bf, 1.0)
            zrow = gp.tile([1, PITCH], BF16, name="zrow")
            nc.vector.memset(zrow, 0.0)
            eps_sb = gp.tile([128, 1], F32, name="eps_sb")
            nc.vector.memset(eps_sb, EPS)
            b2_sb = gp.tile([128, 1], F32, name="b2_sb")
            nc.vector.memset(b2_sb, b2)

            mask_dram = dpool.tile([1, PITCH * PITCH], BF16, name="mask_dram")
            pool_x = es.enter_context(tc.tile_pool(name="px", bufs=1))

            with tc.tile_pool(name="pmsk", bufs=1) as pmsk:
             with tc.tile_pool(name="srp", bufs=9) as srp:
              # ============== Phase 1: conv1 + spatial mask ==============
              with tc.tile_pool(name="pms", bufs=1) as pms:
                mask_sb = pms.tile([1, H, PITCH], BF16, name="mask_sb")
                nc.gpsimd.memset(mask_sb, 0.0)
                mask3 = mask_sb  # [1, 88, 90]
                with tc.tile_pool(name="pw1", bufs=1) as pw1, \
                     tc.tile_pool(name="ps1", bufs=3, space="PSUM") as ps1, \
                     tc.tile_pool(name="ps1m", bufs=2, space="PSUM") as ps1m, \
                     tc.tile_pool(name="ev1", bufs=2) as ev:
                    if maxphase < 1:
                        return
                    # weights first: the first conv matmul gates on these
                    w1v = w1_d.rearrange("o (t p) c -> t p o c", p=128)
                    w1_sb = [pw1.tile([128, 9, C], BF16, name=f"w1_{t}",
                                      tag=f"w1_{t}") for t in range(4)]
                    x_sb = [pool_x.tile([128, H, PITCH], BF16, name=f"x{t}",
                                        tag=f"x{t}") for t in range(4)]
                    xv = x_d.rearrange("(t p) (h q) -> t p h q", p=128, q=PITCH)
                    nc.sync.dma_start(out=w1_sb[0], in_=w1v[0])
                    for t in range(4):
                        nc.sync.dma_start(out=x_sb[t][:, 0:22, :],
                                          in_=xv[t][:, 0:22, :])
                    for t in range(1, 4):
                        nc.sync.dma_start(out=w1_sb[t], in_=w1v[t])
                    nc.sync.dma_start(out=cb_sb, in_=cb_d[:, :])
                    for t in range(2):
                        nc.sync.dma_start(out=kq_sb[:, t, :],
                                          in_=kq_d.rearrange("(t p) q -> t p q", p=128)[t])
                    for t in range(2):
                        nc.sync.dma_start(out=vw2_sb[:, t, :],
                                          in_=vw2_d.rearrange("(t p) q -> t p q", p=128)[t])
                    nc.sync.dma_start(out=vbr_sb, in_=vbr_d[:, :])
                    for t in range(2):
                        nc.sync.dma_start(out=w2_sb[:, t, :],
                                          in_=w2_d.rearrange("(t p) q -> t p q", p=128)[t])
                    for rc in range(1, 4):
                        rs = slice(rc * 22, (rc + 1) * 22)
                        for t in range(4):
                            nc.sync.dma_start(out=x_sb[t][:, rs, :],
                                              in_=xv[t][:, rs, :])

                    def xview(ci, rlo, rhi, dx):
                        return x_sb[ci][:, rlo:rhi, dx:dx + W]

                    for y0, nr in BLOCKS:
                        nn = nr * W
                        h1b = []
                        for cb_i in range(2):
                            ps = ps1.tile([128, nr, W], F32, name="c1ps", tag="c1ps")
                            psf = ps.rearrange("p r w -> p (r w)")
                            _conv3x3(nc, psf,
                                     lambda o, ci, cb_i=cb_i:
                                         w1_sb[ci][:, o, cb_i * 128:(cb_i + 1) * 128],
                                     xview, y0, nr, 4, stop_last=True)
                            h1t = ev.tile([128, nn], BF16, name="h1t", tag=f"h1t{cb_i}")
                            nc.scalar.activation(h1t, psf, AF.Relu,
                                                 bias=cbc(CB_T1, cb_i),
                                                 scale=cbc(CB_S1, cb_i))
                            h1b.append(h1t)
                        mps = ps1m.tile([1, nn], F32, name="mps", tag="mps")
                        for cb_i in range(2):
                            nc.tensor.matmul(mps, w2_sb[:, cb_i, :], h1b[cb_i],
                                             start=(cb_i == 0), stop=(cb_i == 1))
                        nc.scalar.activation(mask3[:, y0:y0 + nr, 1:89], mps,
                                             AF.Sigmoid, bias=b2_sb[0:1, :], scale=1.0)
                # prefetch first r2d srw chunks ahead of the mask-gated DMAs
                preload = {}
                for grp, ci in [(0, 0), (0, 1), (1, 0), (1, 1), (2, 0), (2, 1), (3, 0), (3, 1)]:
                    wp = srp.tile([128, 16, C], BF16, name="wch", tag="wch")
                    nc.sync.dma_start(out=wp, in_=srw_d[1].rearrange(
                        "(g o) (t p) c -> g t p o c", g=4, p=128)[grp, ci])
                    preload[(grp, ci)] = wp
                # mask -> zero-padded 90x90 in DRAM
                nc.sync.dma_start(out=mask_dram[:, 0:PITCH], in_=zrow)
                nc.sync.dma_start(out=mask_dram[:, 89 * PITCH:], in_=zrow)
                nc.sync.dma_start(out=mask_dram[:, PITCH:89 * PITCH],
                                  in_=mask_sb.rearrange("o h q -> o (h q)"))
                if dbg:
                    nc.sync.dma_start(out=dbg_d["mask"][:, :],
                                      in_=mask_sb.rearrange("o h q -> o (h q)"))
              if maxphase < 2:
                  return
              with tc.tile_pool(name="pmb", bufs=1) as pmb:
                  mask_b = pmb.tile([128, H, W], BF16, name="mask_b")
                  m90 = mask_dram.rearrange("o (h q) -> o h q", q=PITCH)
                  nc.sync.dma_start(out=mask_b,
                                    in_=m90[:, 1:89, 1:89].to_broadcast([128, H, W]))
                  msk_sb = []
                  for t in range(2):
                      mt = pmsk.tile([128, H, W], BF16, name=f"msk{t}", tag=f"msk{t}")
                      nc.vector.tensor_tensor(out=mt, in0=x_sb[2 + t][:, :, 1:89],
                                              in1=mask_b, op=ALU.mult)
                      msk_sb.append(mt)
                  if dbg:
                      nc.sync.dma_start(out=dbg_d["msk0"][:, :],
                                        in_=msk_sb[0].rearrange("p a b -> p (a b)"))

              # ====== Phase 2: sr-conv + channel-LN + K / V^T (r2d then d2r) ======
              kvs = {}
              with tc.tile_pool(name="ps2", bufs=1, space="PSUM") as ps2, \
                   tc.tile_pool(name="ps2s", bufs=1, space="PSUM") as ps2s, \
                   tc.tile_pool(name="ev2", bufs=2) as ev:
                  for di in (1, 0):
                      if di == 0:
                          kvf_view = lambda ci, dy, dx: msk_sb[ci][:, dy::RR, dx::RR]
                      else:
                          kvf_view = lambda ci, dy, dx: \
                              x_sb[ci][:, dy::RR, 1 + dx:1 + dx + 81:RR]
                      srps = [ps2.tile([128, M2], F32, name="srps", tag=f"srps{i}")
                              for i in range(2)]
                      for grp in range(4):
                          for ci in range(2):
                              if di == 1 and (grp, ci) in preload:
                                  wch = preload[(grp, ci)]
                              else:
                                  wch = srp.tile([128, 16, C], BF16, name="wch",
                                                 tag="wch")
                                  nc.sync.dma_start(out=wch, in_=srw_d[di].rearrange(
                                      "(g o) (t p) c -> g t p o c", g=4, p=128)[grp, ci])
                              for o in range(16):
                                  off = grp * 16 + o
                                  rhs = kvf_view(ci, off // 8, off % 8)
                                  for cb_i in range(2):
                                      nc.tensor.matmul(
                                          srps[cb_i],
                                          wch[:, o, cb_i * 128:(cb_i + 1) * 128], rhs,
                                          start=(off == 0 and ci == 0),
                                          stop=(off == 63 and ci == 1))
                      kvr = []
                      for cb_i in range(2):
                          kt = ev.tile([128, M2], BF16, name="kvr", tag=f"kvr{cb_i}")
                          nc.scalar.activation(kt, srps[cb_i], AF.Identity,
                                               bias=cbc(CB_SRB0 + di, cb_i), scale=1.0)
                          kvr.append(kt)
                          if dbg:
                              nc.sync.dma_start(
                                  out=dbg_d[f"kvr{di}"][cb_i * 128:(cb_i + 1) * 128, :],
                                  in_=kt)
                      mu_ps = ps2s.tile([1, M2], F32, name="mups", tag="mups")
                      sq_ps = ps2s.tile([1, M2], F32, name="sqps", tag="sqps")
                      for cb_i in range(2):
                          sq = ev.tile([128, M2], BF16, name="sqkv", tag="sqkv")
                          nc.vector.tensor_tensor(out=sq, in0=kvr[cb_i], in1=kvr[cb_i],
                                                  op=ALU.mult)
                          nc.tensor.matmul(mu_ps, ones_bf, kvr[cb_i],
                                           start=(cb_i == 0), stop=(cb_i == 1))
                          nc.tensor.matmul(sq_ps, ones_bf, sq,
                                           start=(cb_i == 0), stop=(cb_i == 1))
                      mu = ev.tile([1, M2], F32, name="mukv", tag="mukv")
                      nc.vector.tensor_scalar(mu, mu_ps, 1.0 / C, None, ALU.mult)
                      ms = ev.tile([1, M2], F32, name="mskv", tag="mskv")
                      nc.vector.tensor_scalar(ms, sq_ps, 1.0 / C, None, ALU.mult)
                      mu2 = ev.tile([1, M2], F32, name="mu2kv", tag="mu2kv")
                      nc.vector.tensor_tensor(out=mu2, in0=mu, in1=mu, op=ALU.mult)
                      nc.vector.tensor_tensor(out=ms, in0=ms, in1=mu2, op=ALU.subtract)
                      sd = ev.tile([1, M2], F32, name="sdkv", tag="sdkv")
                      nc.scalar.activation(sd, ms, AF.Sqrt, bias=eps_sb[0:1, :],
                                           scale=1.0)
                      rstd = ev.tile([1, M2], F32, name="rstdkv", tag="rstdkv")
                      nc.vector.reciprocal(rstd, sd)
                      nrm_bf = ev.tile([1, 2, M2], BF16, name="nrmbf", tag="nrmbf")
                      nc.vector.tensor_copy(nrm_bf[:, 0, :], rstd)
                      murm = ev.tile([1, M2], F32, name="murm", tag="murm")
                      nc.vector.tensor_tensor(out=murm, in0=mu, in1=rstd, op=ALU.mult)
                      nc.vector.tensor_copy(nrm_bf[:, 1, :], murm)
                      nrm_dram = dpool.tile([2, M2], BF16, name="nrm_dram",
                                            tag="nrm_dram", bufs=2)
                      nc.sync.dma_start(out=nrm_dram[0:1, :], in_=nrm_bf[:, 0, :])
                      nc.sync.dma_start(out=nrm_dram[1:2, :], in_=nrm_bf[:, 1, :])
                      rstd_b = ev.tile([128, M2], BF16, name="rstdb", tag="rstdb")
                      nc.sync.dma_start(out=rstd_b,
                                        in_=nrm_dram[0:1, :].to_broadcast([128, M2]))
                      mur_b = ev.tile([128, M2], BF16, name="murb", tag="murb")
                      nc.sync.dma_start(out=mur_b,
                                        in_=nrm_dram[1:2, :].to_broadcast([128, M2]))
                      kvn = []
                      for cb_i in range(2):
                          kn = gp.tile([128, M2], BF16, name=f"kvn{di}{cb_i}")
                          nc.vector.tensor_tensor(out=kn, in0=kvr[cb_i], in1=rstd_b,
                                                  op=ALU.mult)
                          nc.vector.tensor_tensor(out=kn, in0=kn, in1=mur_b,
                                                  op=ALU.subtract)
                          kvn.append(kn)
                          if dbg:
                              nc.sync.dma_start(
                                  out=dbg_d[f"kvn{di}"][cb_i * 128:(cb_i + 1) * 128, :],
                                  in_=kn)
                      kps = ps2s.tile([32, M2], F32, name="kps", tag="kps")
                      for cb_i in range(2):
                          nc.tensor.matmul(kps, kq_sb[:, cb_i, di * 64:di * 64 + 32],
                                           kvn[cb_i], start=(cb_i == 0),
                                           stop=(cb_i == 1))
                      k_bf = gp.tile([32, M2], BF16, name=f"k_bf{di}")
                      nc.scalar.activation(
                          k_bf, kps, AF.Identity,
                          bias=cb_sb[0:32, CB_KB0 + 2 * di:CB_KB0 + 2 * di + 1],
                          scale=1.0)
                      vps = ps2.tile([M2, C], F32, name="vps", tag="vps")
                      for cb_i in range(2):
                          nc.tensor.matmul(vps, kvn[cb_i],
                                           vw2_sb[:, cb_i, di * C:(di + 1) * C],
                                           start=(cb_i == 0), stop=False)
                      nc.tensor.matmul(vps, ones1_bf, vbr_sb[:, di * C:(di + 1) * C],
                                       start=False, stop=True)
                      v_bf = gp.tile([M2, C], BF16, name=f"v_bf{di}")
                      vcol = ev.tile([M2, 1], F32, name="vcol", tag="vcol")
                      nc.scalar.activation(v_bf, vps, AF.Identity, accum_out=vcol)
                      vc_bf = gp.tile([M2, 1], BF16, name=f"vc_bf{di}")
                      nc.vector.tensor_copy(vc_bf, vcol)
                      if dbg:
                          nc.sync.dma_start(out=dbg_d[f"k{di}"][:, :], in_=k_bf)
                          nc.sync.dma_start(out=dbg_d[f"v{di}"][:, :], in_=v_bf)
                      kvs[di] = (k_bf, v_bf, vc_bf)

             # ====== Phase 3: attention + LN + residual (r2d then d2r) ======
             if maxphase < 3:
                 return
             pfw_es = ExitStack()
             pfw = pfw_es.enter_context(tc.tile_pool(name="pfw", bufs=1))
             fw_sb = []
             for t in range(4):
                 ft_ = pfw.tile([128, 9, C], BF16, name=f"fw_{t}", tag=f"fw_{t}")
                 nc.sync.dma_start(
                     out=ft_, in_=fw_d.rearrange("o (t p) c -> t p o c", p=128)[t])
                 fw_sb.append(ft_)
             fwm_sb = pfw.tile([9, C], BF16, name="fwm_sb")
             nc.sync.dma_start(out=fwm_sb, in_=fwm_d[:, :])
             im2 = pfw.tile([9, PITCH * PITCH], BF16, name="im2")
             nc.gpsimd.memset(im2, 0.0)
             for dy in range(3):
                 for dx in range(3):
                     j = dy * 3 + dx
                     joff = dy * PITCH + dx
                     nc.sync.dma_start(
                         out=im2[j:j + 1, 0:PITCH * PITCH - joff],
                         in_=mask_dram[:, joff:])
             with tc.tile_pool(name="ps3", bufs=1, space="PSUM") as ps3, \
                  tc.tile_pool(name="ps3n", bufs=1, space="PSUM") as ps3n, \
                  tc.tile_pool(name="ev3", bufs=2) as ev, \
                  tc.tile_pool(name="nump", bufs=1) as num_p, \
                  tc.tile_pool(name="rbp", bufs=2) as rb_p:
                for di in (1, 0):
                    stats_dram = dpool.tile([2, N], F32, name=f"stats_dram{di}",
                                            tag="stats_dram", bufs=2)
                    rmur_dram = dpool.tile([2, N], BF16, name=f"rmur_dram{di}",
                                           tag="rmur_dram", bufs=2)
                    k_bf, v_bf, vc_bf = kvs[di]
                    num_sb = [num_p.tile([128, N], BF16, name=f"num{di}{cb_i}",
                                         tag=f"num{cb_i}") for cb_i in range(2)]

                    def emit_stats_apply(di, ch, num_sb=num_sb,
                                         stats_dram=stats_dram, rmur_dram=rmur_dram):
                        c0 = ch * CHUNK_ROWS * W
                        cn = CHUNK_ROWS * W
                        mu_t = ev.tile([121, 16], F32, name="mu_t", tag="mu_t")
                        nc.sync.dma_start(
                            out=mu_t,
                            in_=stats_dram[0, c0:c0 + cn].rearrange("(p j) -> p j", j=16))
                        ms_t = ev.tile([121, 16], F32, name="ms_t", tag="ms_t")
                        nc.sync.dma_start(
                            out=ms_t,
                            in_=stats_dram[1, c0:c0 + cn].rearrange("(p j) -> p j", j=16))
                        mu2_t = ev.tile([121, 16], F32, name="mu2_t", tag="mu2_t")
                        nc.vector.tensor_tensor(out=mu2_t, in0=mu_t, in1=mu_t,
                                                op=ALU.mult)
                        nc.vector.tensor_tensor(out=ms_t, in0=ms_t, in1=mu2_t,
                                                op=ALU.subtract)
                        sd_t = ev.tile([121, 16], F32, name="sd_t", tag="sd_t")
                        nc.scalar.activation(sd_t, ms_t, AF.Sqrt,
                                             bias=eps_sb[0:121, :], scale=1.0)
                        r_t = ev.tile([121, 16], F32, name="r_t", tag="r_t")
                        nc.vector.reciprocal(r_t, sd_t)
                        rm_bf = ev.tile([121, 2, 16], BF16, name="rm_bf", tag="rm_bf")
                        nc.vector.tensor_copy(rm_bf[:, 0, :], r_t)
                        nc.vector.tensor_tensor(out=mu_t, in0=mu_t, in1=r_t,
                                                op=ALU.mult)
                        nc.vector.tensor_copy(rm_bf[:, 1, :], mu_t)
                        nc.sync.dma_start(
                            out=rmur_dram[:, c0:c0 + cn]
                            .rearrange("t (p j) -> p t j", j=16), in_=rm_bf)
                        r_b = rb_p.tile([128, cn], BF16, name="r_b", tag="r_b")
                        nc.sync.dma_start(
                            out=r_b,
                            in_=rmur_dram[0:1, c0:c0 + cn].to_broadcast([128, cn]))
                        mur_b = rb_p.tile([128, cn], BF16, name="mur_b", tag="mur_b")
                        nc.sync.dma_start(
                            out=mur_b,
                            in_=rmur_dram[1:2, c0:c0 + cn].to_broadcast([128, cn]))
                        for cb_i in range(2):
                            seg = num_sb[cb_i][:, c0:c0 + cn]
                            nc.vector.tensor_tensor(out=seg, in0=seg, in1=r_b,
                                                    op=ALU.mult)
                            nc.vector.tensor_tensor(out=seg, in0=seg, in1=mur_b,
                                                    op=ALU.subtract)
                            nc.scalar.activation(seg, seg, AF.Identity,
                                                 bias=cbc(CB_NB0 + 2 * di, cb_i),
                                                 scale=cbc(CB_NG0 + 2 * di, cb_i))
                            rows = slice(ch * CHUNK_ROWS, (ch + 1) * CHUNK_ROWS)
                            nc.vector.tensor_tensor(
                                out=x_sb[2 * di + cb_i][:, rows, 1:89],
                                in0=num_sb[cb_i]
                                .rearrange("p (h w) -> p h w", w=W)[:, rows, :],
                                in1=x_sb[2 * di + cb_i][:, rows, 1:89], op=ALU.add)

                    TRIG = {4: 0, 8: 1, 13: 2, 17: 3}
                    for bi, (y0, nr) in enumerate(BLOCKS):
                        nn = nr * W
                        qps = ps3n.tile([32, nn], F32, name="qps", tag="qps")
                        for ci in range(2):
                            rhs = (msk_sb[ci][:, y0:y0 + nr, :] if di == 1
                                   else x_sb[ci][:, y0:y0 + nr, 1:89])
                            nc.tensor.matmul(qps,
                                             kq_sb[:, ci, di * 64 + 32:di * 64 + 64],
                                             rhs, start=(ci == 0), stop=(ci == 1))
                        q_bf = ev.tile([32, nn], BF16, name="q_bf", tag="q_bf")
                        nc.scalar.activation(
                            q_bf, qps, AF.Identity,
                            bias=cb_sb[0:32, CB_QB0 + 2 * di:CB_QB0 + 2 * di + 1],
                            scale=1.0)
                        sps = ps3.tile([M2, nn], F32, name="sps", tag="sps")
                        nc.tensor.matmul(sps, k_bf, q_bf, start=True, stop=True)
                        e_bf = ev.tile([M2, nn], BF16, name="e_bf", tag="e_bf")
                        nc.scalar.activation(e_bf, sps, AF.Exp)
                        mu_ps = ps3n.tile([1, nn], F32, name="amups", tag="astps",
                                          bufs=2)
                        nc.tensor.matmul(mu_ps, vc_bf, e_bf, start=True, stop=True)
                        sq_ps = ps3n.tile([1, nn], F32, name="asqps", tag="astps",
                                          bufs=2)
                        for cb_i in range(2):
                            nps = ps3.tile([128, nn], F32, name="nps",
                                           tag=f"nps{cb_i}", bufs=2)
                            nc.tensor.matmul(nps,
                                             v_bf[:, cb_i * 128:(cb_i + 1) * 128],
                                             e_bf, start=True, stop=True)
                            nc.vector.tensor_copy(num_sb[cb_i][:, y0 * W:y0 * W + nn],
                                                  nps)
                            nsq = ev.tile([128, nn], BF16, name="nsq", tag="nsq")
                            if cb_i == 0:
                                nc.scalar.activation(nsq, nps, AF.Square)
                            else:
                                segq = num_sb[cb_i][:, y0 * W:y0 * W + nn]
                                nc.vector.tensor_tensor(out=nsq, in0=segq, in1=segq,
                                                        op=ALU.mult)
                            nc.tensor.matmul(sq_ps, ones_bf, nsq,
                                             start=(cb_i == 0), stop=(cb_i == 1))
                        mrow = ev.tile([1, nn], F32, name="mrow", tag="mrow")
                        nc.vector.tensor_scalar(mrow, mu_ps, 1.0 / C, None, ALU.mult)
                        nc.sync.dma_start(out=stats_dram[0:1, y0 * W:y0 * W + nn],
                                          in_=mrow)
                        srow = ev.tile([1, nn], F32, name="srow", tag="srow")
                        nc.scalar.activation(srow, sq_ps, AF.Identity, scale=1.0 / C)
                        nc.sync.dma_start(out=stats_dram[1:2, y0 * W:y0 * W + nn],
                                          in_=srow)

                    if dbg:
                        for cb_i in range(2):
                            nc.sync.dma_start(
                                out=dbg_d[f"num{di}"][cb_i * 128:(cb_i + 1) * 128, :],
                                in_=num_sb[cb_i])
                        for ch in range(4):
                            emit_stats_apply(di, ch)
                        nc.sync.dma_start(out=dbg_d[f"rm{di}"][:, :], in_=rmur_dram)
                    else:
                        for ch in range(4):
                            emit_stats_apply(di, ch)

             # ================= Phase 4: conv2 =================
             # ================= Phase 4: conv2 =================
             if maxphase < 4:
                 return
             enh_list = x_sb
             with tc.tile_pool(name="ps4", bufs=4, space="PSUM") as ps4, \
                  tc.tile_pool(name="ev4", bufs=2) as ev:
                 im2v = im2.rearrange("o (h q) -> o h q", q=PITCH)

                 for y0, nr in BLOCKS:
                     nn = nr * W
                     for cb_i in range(2):
                         ps = ps4.tile([128, nr, W], F32, name="c2ps", tag="c2ps")
                         psf = ps.rearrange("p r w -> p (r w)")
                         _conv3x3(nc, psf,
                                  lambda o, ci, cb_i=cb_i:
                                      fw_sb[ci][:, o, cb_i * 128:(cb_i + 1) * 128],
                                  lambda ci, rlo, rhi, dx:
                                      enh_list[ci][:, rlo:rhi, dx:dx + W],
                                  y0, nr, 4, stop_last=False, ci_order=[2, 3, 0, 1])
                         nc.tensor.matmul(
                             psf, fwm_sb[:, cb_i * 128:(cb_i + 1) * 128],
                             im2v[:, y0:y0 + nr, 0:W], start=False, stop=True)
                         o_t = ev.tile([128, nn], F32, name="o_t", tag="o_t")
                         nc.scalar.activation(o_t, psf, AF.Relu,
                                              bias=cbc(CB_FT, cb_i),
                                              scale=cbc(CB_FS, cb_i))
                         nc.sync.dma_start(
                             out=out_d[cb_i * 128:(cb_i + 1) * 128,
                                       y0 * W:y0 * W + nn],
                             in_=o_t)
             pfw_es.close()
    nc.finalize()
    return nc


def kernel(**inputs):
    in_maps, b2 = _prep(inputs)
    key = ("nc", round(b2, 9))
    if key not in _CACHE:
        nc = bacc.Bacc("TRN2", target_bir_lowering=False, debug=False)
        _build(nc, b2)
        _CACHE[key] = nc
    nc = _CACHE[key]
    res = run_bass_kernel_spmd(nc, in_maps, list(range(B)))
    return np.stack([np.asarray(res.results[i]["out"], np.float32).reshape(C, H, W)
                     for i in range(B)])



# revision 3
# speedup vs baseline: 1.2690x; 1.2690x over previous
"""Trainium2 Bass kernel for nn_BiDirectionalFusionModule.

Pure batch data-parallelism: 8 samples -> 8 NeuronCores, each core runs the
full module for one sample.

v2: the big matmuls run in fp8e4m3 with DoubleRow perf mode (2 contraction
planes per instruction at 0.5 cycles/row -> 4x the bf16 matmul throughput).
Scale convention: weights are pre-scaled x64 and activations x4 before fp8
quantization (keeps lo planes out of the subnormal range); the resulting
x256 on every PSUM is folded into the evacuation scales.

 - conv1 (512->256 3x3): single-term fp8 DR (error damped through the
   sigmoid mask path; ~0.3% end-to-end).
 - fusion conv (512->256 3x3 + mask channel): 3-term hi/lo fp8 DR
   (Wh*Xh + Wh*Xl + Wl*Xh; the dropped Wl*Xl term is ~0.07%). The mask
   channel stays bf16 (fwm pre-scaled x256 to share the PSUM).
 - spatial-reduction convs and Q projections: single-term fp8 DR.
 - LN-variance row reduction: (num/32)^2 in fp8, ones-DR matmul.
 - attention scores / A@V / mu-reduction and all small matmuls stay bf16.

Host-side folds: BN affine -> post-conv scale/bias; LN affine + softmax scale
-> K/V 1x1 conv weights; clip(gamma) -> post-attention LN affine. The softmax
denominator and max-subtraction cancel inside the channel-LayerNorm that
follows each attention output, so softmax becomes a bare exp.

SBUF: the four fp8 [128,2,88,90] scratch slots are shared by tag reuse —
x8r/msk8 (inputs of phases 1-3) are overwritten by the enh hi/lo planes as
each direction's apply runs (WAR deps are tracked by the tile framework).
The bf16 residual base streams back from DRAM per apply chunk.
"""
import numpy as np
import ml_dtypes
from contextlib import ExitStack

import concourse.bass as bass
from concourse import bacc
import concourse.tile as tile
import concourse.mybir as mybir
from concourse.bass_utils import run_bass_kernel_spmd

F32 = mybir.dt.float32
BF16 = mybir.dt.bfloat16
F8 = mybir.dt.float8e4
AF = mybir.ActivationFunctionType
ALU = mybir.AluOpType
DR = mybir.MatmulPerfMode.DoubleRow
BF = ml_dtypes.bfloat16
F8NP = ml_dtypes.float8_e4m3

B, C, H, W = 8, 256, 88, 88
RR = 8
HR = H // RR                # 11
M2 = HR * HR                # 121
N = H * W                   # 7744
PITCH = 90
EPS = 1e-5
CQ = C // 8                 # 32

SW = 64.0                   # weight fp8 prescale
SX = 4.0                    # activation fp8 prescale
SWX = SW * SX               # folded into PSUM evacuation scales
SQS = 1.0 / 32.0            # num prescale inside Square
BLOCKS = [(i * 5, 5) for i in range(17)] + [(85, 3)]
CHUNK_ROWS = 11             # apply-phase chunking: 8 chunks of 11 rows
NCH = H // CHUNK_ROWS       # 8
STJ = CHUNK_ROWS * W // M2  # 8  (968 = 121*8 stat-tile cols)

(CB_S1, CB_T1, CB_SRB0, CB_SRB1, CB_NG0, CB_NB0, CB_NG1, CB_NB1, CB_FS, CB_FT,
 CB_KB0, CB_QB0, CB_KB1, CB_QB1) = range(14)

_CACHE = {}


def _q8(x, s):
    return (np.asarray(x, np.float32) * s).astype(F8NP)


def _prep(inputs):
    ii = {k: np.asarray(v, dtype=np.float32) for k, v in inputs.items()}
    scale = float(CQ) ** -0.5

    def fold_bn(g, be, m, v):
        s = g / np.sqrt(v + EPS)
        return s, (0.0 - m) * s + be

    def pack_dr(w):  # [9, 512, cout] -> [128, pair, plane, 9, cout] (no quant)
        o, cin, co = w.shape
        return w.reshape(o, 2, 2, 128, co).transpose(3, 1, 2, 0, 4)

    # conv1 weights, fp8 DR layout
    w1 = ii['sm_w1'].transpose(2, 3, 1, 0).reshape(9, 2 * C, C)
    w1_8 = _q8(pack_dr(w1), SW)
    s1, t1 = fold_bn(ii['sm_g1'], ii['sm_be1'], ii['sm_m1'], ii['sm_v1'])
    t1 = t1 + ii['sm_b1'] * s1
    w2T = ii['sm_w2'][:, :, 0, 0].T.astype(BF)
    b2 = float(ii['sm_b2'][0])

    # fusion conv weights: hi/lo fp8 planes + bf16 mask column (x256)
    fw = pack_dr(ii['fus_w'][:, :2 * C].transpose(2, 3, 1, 0).reshape(9, 2 * C, C))
    fwh_8 = _q8(fw, SW)
    fwl_8 = _q8(fw - fwh_8.astype(np.float32) / SW, SW)
    fwm = (ii['fus_w'][:, 2 * C, :, :].transpose(1, 2, 0).reshape(9, C)
           * SWX).astype(BF)
    fs, ft = fold_bn(ii['fus_g'], ii['fus_be'], ii['fus_m'], ii['fus_v'])
    ft = ft + ii['fus_b'] * fs

    dirs = {}
    for di, pfx in enumerate(('d2r', 'r2d')):
        g = ii[pfx + '_ln_g']; bl = ii[pfx + '_ln_b']
        kw = ii[pfx + '_k_w'][:, :, 0, 0]; kb = ii[pfx + '_k_b']
        vw = ii[pfx + '_v_w'][:, :, 0, 0]; vb = ii[pfx + '_v_b']
        qw = ii[pfx + '_q_w'][:, :, 0, 0]; qb = ii[pfx + '_q_b']
        gamma = float(np.clip(ii[pfx + '_gamma'], 0.0, 1.0)[0])
        # sr conv fp8 [grp, k=128, plane, off16, cout]
        srw = ii[pfx + '_sr_w'].transpose(2, 3, 1, 0).reshape(64, C, C)
        srw8 = _q8(srw.reshape(4, 16, 2, 128, C).transpose(0, 3, 2, 1, 4), SW)
        dirs[di] = dict(
            srw8=np.ascontiguousarray(srw8),
            srb=ii[pfx + '_sr_b'],
            kwT=(scale * kw * g[None, :]).T.astype(BF),
            kb=scale * (kb + kw @ bl),
            qw8=_q8(qw.T, SW), qb=qb,
            vwN=(vw * g[None, :]).T.astype(BF),
            vb=(vb + vw @ bl).astype(BF),
            ng=gamma * ii[pfx + '_norm_g'],
            nb=gamma * ii[pfx + '_norm_b'],
        )

    cb = np.zeros((C, 14), np.float32)
    cb[:, CB_S1] = s1 / SWX; cb[:, CB_T1] = t1
    cb[:, CB_FS] = fs / SWX; cb[:, CB_FT] = ft
    for di in range(2):
        d = dirs[di]
        cb[:, CB_SRB0 + di] = d['srb']
        cb[:, CB_NG0 + 2 * di] = SX * d['ng']
        cb[:, CB_NB0 + 2 * di] = SX * d['nb']
        cb[:CQ, CB_KB0 + 2 * di] = d['kb']
        cb[:CQ, CB_QB0 + 2 * di] = d['qb']
    cbp = np.zeros((128, 28), np.float32)
    cbp[:, 0:14] = cb[0:128]; cbp[:, 14:28] = cb[128:256]

    kq = np.zeros((C, 128), BF)
    kq[:, 0:32] = dirs[0]['kwT']; kq[:, 64:96] = dirs[1]['kwT']
    kq8 = np.zeros((128, 2, 64), F8NP)
    for pl in range(2):
        kq8[:, pl, 0:32] = dirs[0]['qw8'][pl * 128:(pl + 1) * 128]
        kq8[:, pl, 32:64] = dirs[1]['qw8'][pl * 128:(pl + 1) * 128]
    vw2 = np.concatenate([dirs[0]['vwN'], dirs[1]['vwN']], axis=1)
    vbr = np.concatenate([dirs[0]['vb'], dirs[1]['vb']])[None, :]

    shared = dict(w1=np.ascontiguousarray(w1_8), w2=w2T,
                  fwh=np.ascontiguousarray(fwh_8),
                  fwl=np.ascontiguousarray(fwl_8), fwm=fwm, cb=cbp,
                  kq=kq, kq8=kq8,
                  vw2=np.ascontiguousarray(vw2), vbr=np.ascontiguousarray(vbr),
                  srw0=dirs[0]['srw8'], srw1=dirs[1]['srw8'])

    rgb = ii['f_rgb']; dep = ii['f_depth']
    in_maps = []
    for i in range(B):
        xr = np.zeros((C, H, PITCH), np.float32)
        xr[:, :, 1:89] = rgb[i]
        xd = np.zeros((C, H, PITCH), np.float32)
        xd[:, :, 1:89] = dep[i]
        m = dict(shared)
        # bf16 copy at x4 (residual base + mask multiply input)
        xb = np.concatenate([xr, xd], 0) * SX
        m['x'] = np.ascontiguousarray(xb.astype(BF).reshape(2 * C, H * PITCH))
        m['x8r'] = np.ascontiguousarray(
            _q8(xr, SX).reshape(2, 128, H * PITCH).transpose(1, 0, 2))
        m['x8d'] = np.ascontiguousarray(
            _q8(xd, SX).reshape(2, 128, H * PITCH).transpose(1, 0, 2))
        in_maps.append(m)
    return in_maps, b2


def _conv3x3_dr(nc, psum, lhsT_of, rhs_of, y0, nr, n_slot, stop_last):
    """Shifted DR matmuls accumulating into psum[128, nr*W].
    lhsT_of(off_idx, slot) -> [128,2,128] AP; rhs_of(slot, rlo, rhi, dx) ->
    [128,2,rows,W] AP. dy==1 offsets first so the initial matmul covers the
    full region."""
    plan = []
    for dy, dx in [(1, 0), (1, 1), (1, 2), (0, 0), (0, 1), (0, 2),
                   (2, 0), (2, 1), (2, 2)]:
        s = dy - 1
        ylo = max(y0, -s); yhi = min(y0 + nr, H - s)
        if ylo >= yhi:
            continue
        for sl in range(n_slot):
            plan.append((dy * 3 + dx, sl, s, ylo, yhi))
    for i, (o, sl, s, ylo, yhi) in enumerate(plan):
        out = psum if (ylo == y0 and yhi == y0 + nr) else \
            psum[:, (ylo - y0) * W:(yhi - y0) * W]
        nc.tensor.matmul(out, lhsT_of(o, sl), rhs_of(sl, ylo + s, yhi + s, o % 3),
                         start=(i == 0), stop=(stop_last and i == len(plan) - 1),
                         perf_mode=DR)


def _build(nc, b2, dbg=False, maxphase=4):
    x_d = nc.dram_tensor("x", [2 * C, H * PITCH], BF16, kind="ExternalInput")
    x8r_d = nc.dram_tensor("x8r", [128, 2, H * PITCH], F8, kind="ExternalInput")
    x8d_d = nc.dram_tensor("x8d", [128, 2, H * PITCH], F8, kind="ExternalInput")
    w1_d = nc.dram_tensor("w1", [128, 2, 2, 9, C], F8, kind="ExternalInput")
    w2_d = nc.dram_tensor("w2", [C, 1], BF16, kind="ExternalInput")
    fwh_d = nc.dram_tensor("fwh", [128, 2, 2, 9, C], F8, kind="ExternalInput")
    fwl_d = nc.dram_tensor("fwl", [128, 2, 2, 9, C], F8, kind="ExternalInput")
    fwm_d = nc.dram_tensor("fwm", [9, C], BF16, kind="ExternalInput")
    cb_d = nc.dram_tensor("cb", [128, 28], F32, kind="ExternalInput")
    kq_d = nc.dram_tensor("kq", [C, 128], BF16, kind="ExternalInput")
    kq8_d = nc.dram_tensor("kq8", [128, 2, 64], F8, kind="ExternalInput")
    vw2_d = nc.dram_tensor("vw2", [C, 2 * C], BF16, kind="ExternalInput")
    vbr_d = nc.dram_tensor("vbr", [1, 2 * C], BF16, kind="ExternalInput")
    srw_d = [nc.dram_tensor("srw0", [4, 128, 2, 16, C], F8, kind="ExternalInput"),
             nc.dram_tensor("srw1", [4, 128, 2, 16, C], F8, kind="ExternalInput")]
    out_d = nc.dram_tensor("out", [C, N], F32, kind="ExternalOutput")
    dbg_d = {}
    if dbg:
        for nm, shp in [("mask", [1, H * PITCH]), ("msk0", [128, N]),
                        ("kvr0", [C, M2]), ("kvr1", [C, M2]),
                        ("k0", [32, M2]), ("k1", [32, M2]),
                        ("v0", [M2, C]), ("v1", [M2, C]),
                        ("num0", [C, N]), ("num1", [C, N]),
                        ("ehi", [128, 4 * H * PITCH]),
                        ("elo", [128, 4 * H * PITCH])]:
            dbg_d[nm] = nc.dram_tensor("dbg_" + nm, shp, BF16, kind="ExternalOutput")

    with tile.TileContext(nc) as tc:
        es = ExitStack()
        with es, tc.tile_pool(name="dram", bufs=1, space="DRAM") as dpool:
            gp = es.enter_context(tc.tile_pool(name="gp", bufs=1))
            scr = es.enter_context(tc.tile_pool(name="scr", bufs=1, side="right"))

            cb_sb = gp.tile([128, 28], F32, name="cb_sb")

            def cbc(col, half):
                return cb_sb[:, half * 14 + col:half * 14 + col + 1]

            kq_sb = gp.tile([128, 2, 128], BF16, name="kq_sb")
            kq8_sb = gp.tile([128, 2, 64], F8, name="kq8_sb")
            vw2_sb = gp.tile([128, 2, 2 * C], BF16, name="vw2_sb")
            vbr_sb = gp.tile([1, 2 * C], BF16, name="vbr_sb")
            w2_sb = gp.tile([128, 2, 1], BF16, name="w2_sb")
            ones_bf = gp.tile([128, 1], BF16, name="ones_bf")
            nc.vector.memset(ones_bf, 1.0)
            ones8 = gp.tile([128, 2, 1], F8, name="ones8")
            nc.vector.memset(ones8, 1.0)
            ones1_bf = gp.tile([1, M2], BF16, name="ones1_bf")
            nc.vector.memset(ones1_bf, 1.0)
            zrow = gp.tile([1, PITCH], BF16, name="zrow")
            nc.vector.memset(zrow, 0.0)
            eps_sb = gp.tile([128, 1], F32, name="eps_sb")
            nc.vector.memset(eps_sb, EPS)
            b2_sb = gp.tile([128, 1], F32, name="b2_sb")
            nc.vector.memset(b2_sb, b2)

            mask_dram = dpool.tile([1, PITCH * PITCH], BF16, name="mask_dram")

            # fp8 scratch slots (tag-shared): x8r -> ehi0, msk8 -> ehi1
            x8r = scr.tile([128, 2, H, PITCH], F8, name="x8r", tag="scrA")
            msk8 = scr.tile([128, 2, H, PITCH], F8, name="msk8", tag="scrB")

            preload = {}
            with tc.tile_pool(name="srp", bufs=5) as srp:
              # ============== Phase 1: conv1 + spatial mask ==============
              with tc.tile_pool(name="pms", bufs=1) as pms:
                mask_sb = pms.tile([1, H, PITCH], BF16, name="mask_sb")
                nc.gpsimd.memset(mask_sb, 0.0)
                mask3 = mask_sb  # [1, 88, 90]
                with tc.tile_pool(name="pw1", bufs=1) as pw1, \
                     tc.tile_pool(name="ps1", bufs=3, space="PSUM") as ps1, \
                     tc.tile_pool(name="ps1m", bufs=2, space="PSUM") as ps1m, \
                     tc.tile_pool(name="ev1", bufs=2) as ev:
                    if maxphase < 1:
                        return
                    # weights first: the first conv matmul gates on these
                    w1_sb = pw1.tile([128, 2, 2, 9, C], F8, name="w1_sb")
                    nc.sync.dma_start(out=w1_sb, in_=w1_d[:, :, :, :, :])
                    x8d = pw1.tile([128, 2, H, PITCH], F8, name="x8d")
                    x8rv = x8r_d.rearrange("p t (h q) -> p t h q", q=PITCH)
                    x8dv = x8d_d.rearrange("p t (h q) -> p t h q", q=PITCH)
                    for rc in range(4):
                        rs = slice(rc * 22, (rc + 1) * 22)
                        nc.sync.dma_start(out=x8r[:, :, rs, :], in_=x8rv[:, :, rs, :])
                        nc.sync.dma_start(out=x8d[:, :, rs, :], in_=x8dv[:, :, rs, :])
                    nc.sync.dma_start(out=cb_sb, in_=cb_d[:, :])
                    for t in range(2):
                        nc.sync.dma_start(out=kq_sb[:, t, :],
                                          in_=kq_d.rearrange("(t p) q -> t p q", p=128)[t])
                    nc.sync.dma_start(out=kq8_sb, in_=kq8_d[:, :, :])
                    for t in range(2):
                        nc.sync.dma_start(out=vw2_sb[:, t, :],
                                          in_=vw2_d.rearrange("(t p) q -> t p q", p=128)[t])
                    nc.sync.dma_start(out=vbr_sb, in_=vbr_d[:, :])
                    for t in range(2):
                        nc.sync.dma_start(out=w2_sb[:, t, :],
                                          in_=w2_d.rearrange("(t p) q -> t p q", p=128)[t])
                    # depth bf16 (x4) tiles, for the mask multiply
                    xv = x_d.rearrange("(t p) (h q) -> t p h q", p=128, q=PITCH)
                    xb_dep = [pw1.tile([128, H, PITCH], BF16, name=f"xbd{t}")
                              for t in range(2)]
                    for t in range(2):
                        nc.sync.dma_start(out=xb_dep[t], in_=xv[2 + t])

                    x8p = [x8r, x8d]

                    def rhs1(sl, rlo, rhi, dx):
                        return x8p[sl][:, :, rlo:rhi, dx:dx + W]

                    for y0, nr in BLOCKS:
                        nn = nr * W
                        h1b = []
                        for cb_i in range(2):
                            ps = ps1.tile([128, nr, W], F32, name="c1ps", tag="c1ps")
                            psf = ps.rearrange("p r w -> p (r w)")
                            _conv3x3_dr(nc, psf,
                                        lambda o, sl, cb_i=cb_i:
                                            w1_sb[:, sl, :, o,
                                                  cb_i * 128:(cb_i + 1) * 128],
                                        rhs1, y0, nr, 2, stop_last=True)
                            h1t = ev.tile([128, nn], BF16, name="h1t", tag=f"h1t{cb_i}")
                            nc.scalar.activation(h1t, psf, AF.Relu,
                                                 bias=cbc(CB_T1, cb_i),
                                                 scale=cbc(CB_S1, cb_i))
                            h1b.append(h1t)
                        mps = ps1m.tile([1, nn], F32, name="mps", tag="mps")
                        for cb_i in range(2):
                            nc.tensor.matmul(mps, w2_sb[:, cb_i, :], h1b[cb_i],
                                             start=(cb_i == 0), stop=(cb_i == 1))
                        nc.scalar.activation(mask3[:, y0:y0 + nr, 1:89], mps,
                                             AF.Sigmoid, bias=b2_sb[0:1, :], scale=1.0)
                    # prefetch r2d srw chunks ahead of the mask-gated DMAs
                    for grp in range(4):
                        wp = srp.tile([128, 2, 16, C], F8, name="wch", tag="wch")
                        nc.sync.dma_start(out=wp, in_=srw_d[1][grp])
                        preload[grp] = wp
                    # mask -> zero-padded 90x90 in DRAM
                    nc.sync.dma_start(out=mask_dram[:, 0:PITCH], in_=zrow)
                    nc.sync.dma_start(out=mask_dram[:, 89 * PITCH:], in_=zrow)
                    nc.sync.dma_start(out=mask_dram[:, PITCH:89 * PITCH],
                                      in_=mask_sb.rearrange("o h q -> o (h q)"))
                    if dbg:
                        nc.sync.dma_start(out=dbg_d["mask"][:, :],
                                          in_=mask_sb.rearrange("o h q -> o (h q)"))
                    if maxphase >= 2:
                        with tc.tile_pool(name="pmb", bufs=1) as pmb:
                            mask_b = pmb.tile([128, H, W], BF16, name="mask_b")
                            m90 = mask_dram.rearrange("o (h q) -> o h q", q=PITCH)
                            nc.sync.dma_start(
                                out=mask_b,
                                in_=m90[:, 1:89, 1:89].to_broadcast([128, H, W]))
                            for t in range(2):
                                nc.vector.tensor_tensor(
                                    out=msk8[:, t, :, 0:W],
                                    in0=xb_dep[t][:, :, 1:89],
                                    in1=mask_b, op=ALU.mult)
                            if dbg:
                                mb0 = pmb.tile([128, H, W], BF16, name="mb0")
                                nc.vector.tensor_copy(mb0, msk8[:, 0, :, 0:W])
                                nc.sync.dma_start(
                                    out=dbg_d["msk0"][:, :],
                                    in_=mb0.rearrange("p a b -> p (a b)"))
              if maxphase < 2:
                  return

              # ====== Phase 2: sr-conv + channel-LN + K / V^T (r2d then d2r) ======
              kvs = {}
              with tc.tile_pool(name="ps2", bufs=1, space="PSUM") as ps2, \
                   tc.tile_pool(name="ps2s", bufs=1, space="PSUM") as ps2s, \
                   tc.tile_pool(name="ev2", bufs=2) as ev:
                  for di in (1, 0):
                      if di == 0:
                          srrhs = lambda dy, dx: \
                              msk8[:, :, dy::RR, dx:dx + 81:RR]
                      else:
                          srrhs = lambda dy, dx: \
                              x8r[:, :, dy::RR, 1 + dx:1 + dx + 81:RR]
                      srps = [ps2.tile([128, M2], F32, name="srps", tag=f"srps{i}")
                              for i in range(2)]
                      for grp in range(4):
                          if di == 1:
                              wch = preload[grp]
                          else:
                              wch = srp.tile([128, 2, 16, C], F8, name="wch",
                                             tag="wch")
                              nc.sync.dma_start(out=wch, in_=srw_d[di][grp])
                          for o in range(16):
                              off = grp * 16 + o
                              rhs = srrhs(off // 8, off % 8)
                              for cb_i in range(2):
                                  nc.tensor.matmul(
                                      srps[cb_i],
                                      wch[:, :, o, cb_i * 128:(cb_i + 1) * 128],
                                      rhs,
                                      start=(off == 0),
                                      stop=(off == 63), perf_mode=DR)
                      kvr = []
                      for cb_i in range(2):
                          kt = ev.tile([128, M2], BF16, name="kvr", tag=f"kvr{cb_i}")
                          nc.scalar.activation(kt, srps[cb_i], AF.Identity,
                                               bias=cbc(CB_SRB0 + di, cb_i),
                                               scale=1.0 / SWX)
                          kvr.append(kt)
                          if dbg:
                              nc.sync.dma_start(
                                  out=dbg_d[f"kvr{di}"][cb_i * 128:(cb_i + 1) * 128, :],
                                  in_=kt)
                      mu_ps = ps2s.tile([1, M2], F32, name="mups", tag="mups")
                      sq_ps = ps2s.tile([1, M2], F32, name="sqps", tag="sqps")
                      for cb_i in range(2):
                          sq = ev.tile([128, M2], BF16, name="sqkv", tag="sqkv")
                          nc.vector.tensor_tensor(out=sq, in0=kvr[cb_i], in1=kvr[cb_i],
                                                  op=ALU.mult)
                          nc.tensor.matmul(mu_ps, ones_bf, kvr[cb_i],
                                           start=(cb_i == 0), stop=(cb_i == 1))
                          nc.tensor.matmul(sq_ps, ones_bf, sq,
                                           start=(cb_i == 0), stop=(cb_i == 1))
                      mu = ev.tile([1, M2], F32, name="mukv", tag="mukv")
                      nc.vector.tensor_scalar(mu, mu_ps, 1.0 / C, None, ALU.mult)
                      ms = ev.tile([1, M2], F32, name="mskv", tag="mskv")
                      nc.vector.tensor_scalar(ms, sq_ps, 1.0 / C, None, ALU.mult)
                      mu2 = ev.tile([1, M2], F32, name="mu2kv", tag="mu2kv")
                      nc.vector.tensor_tensor(out=mu2, in0=mu, in1=mu, op=ALU.mult)
                      nc.vector.tensor_tensor(out=ms, in0=ms, in1=mu2, op=ALU.subtract)
                      sd = ev.tile([1, M2], F32, name="sdkv", tag="sdkv")
                      nc.scalar.activation(sd, ms, AF.Sqrt, bias=eps_sb[0:1, :],
                                           scale=1.0)
                      rstd = ev.tile([1, M2], F32, name="rstdkv", tag="rstdkv")
                      nc.vector.reciprocal(rstd, sd)
                      nrm_bf = ev.tile([1, 2, M2], BF16, name="nrmbf", tag="nrmbf")
                      nc.vector.tensor_copy(nrm_bf[:, 0, :], rstd)
                      murm = ev.tile([1, M2], F32, name="murm", tag="murm")
                      nc.vector.tensor_tensor(out=murm, in0=mu, in1=rstd, op=ALU.mult)
                      nc.vector.tensor_copy(nrm_bf[:, 1, :], murm)
                      nrm_dram = dpool.tile([2, M2], BF16, name="nrm_dram",
                                            tag="nrm_dram", bufs=2)
                      nc.sync.dma_start(out=nrm_dram[0:1, :], in_=nrm_bf[:, 0, :])
                      nc.sync.dma_start(out=nrm_dram[1:2, :], in_=nrm_bf[:, 1, :])
                      rstd_b = ev.tile([128, M2], BF16, name="rstdb", tag="rstdb")
                      nc.sync.dma_start(out=rstd_b,
                                        in_=nrm_dram[0:1, :].to_broadcast([128, M2]))
                      mur_b = ev.tile([128, M2], BF16, name="murb", tag="murb")
                      nc.sync.dma_start(out=mur_b,
                                        in_=nrm_dram[1:2, :].to_broadcast([128, M2]))
                      kvn = []
                      for cb_i in range(2):
                          kn = gp.tile([128, M2], BF16, name=f"kvn{di}{cb_i}")
                          nc.vector.tensor_tensor(out=kn, in0=kvr[cb_i], in1=rstd_b,
                                                  op=ALU.mult)
                          nc.vector.tensor_tensor(out=kn, in0=kn, in1=mur_b,
                                                  op=ALU.subtract)
                          kvn.append(kn)
                      kps = ps2s.tile([32, M2], F32, name="kps", tag="kps")
                      for cb_i in range(2):
                          nc.tensor.matmul(kps, kq_sb[:, cb_i, di * 64:di * 64 + 32],
                                           kvn[cb_i], start=(cb_i == 0),
                                           stop=(cb_i == 1))
                      k_bf = gp.tile([32, M2], BF16, name=f"k_bf{di}")
                      nc.scalar.activation(
                          k_bf, kps, AF.Identity,
                          bias=cb_sb[0:32, CB_KB0 + 2 * di:CB_KB0 + 2 * di + 1],
                          scale=1.0)
                      vps = ps2.tile([M2, C], F32, name="vps", tag="vps")
                      for cb_i in range(2):
                          nc.tensor.matmul(vps, kvn[cb_i],
                                           vw2_sb[:, cb_i, di * C:(di + 1) * C],
                                           start=(cb_i == 0), stop=False)
                      nc.tensor.matmul(vps, ones1_bf, vbr_sb[:, di * C:(di + 1) * C],
                                       start=False, stop=True)
                      v_bf = gp.tile([M2, C], BF16, name=f"v_bf{di}")
                      vcol = ev.tile([M2, 1], F32, name="vcol", tag="vcol")
                      nc.scalar.activation(v_bf, vps, AF.Identity, accum_out=vcol)
                      vc_bf = gp.tile([M2, 1], BF16, name=f"vc_bf{di}")
                      nc.vector.tensor_copy(vc_bf, vcol)
                      if dbg:
                          nc.sync.dma_start(out=dbg_d[f"k{di}"][:, :], in_=k_bf)
                          nc.sync.dma_start(out=dbg_d[f"v{di}"][:, :], in_=v_bf)
                      kvs[di] = (k_bf, v_bf, vc_bf)

            # ====== Phase 3: attention + LN + residual (r2d then d2r) ======
            if maxphase < 3:
                return
            with tc.tile_pool(name="pfw", bufs=1) as pfw:
                fwh_sb = pfw.tile([128, 2, 2, 9, C], F8, name="fwh_sb")
                nc.sync.dma_start(out=fwh_sb, in_=fwh_d[:, :, :, :, :])
                fwl_sb = pfw.tile([128, 2, 2, 9, C], F8, name="fwl_sb")
                nc.sync.dma_start(out=fwl_sb, in_=fwl_d[:, :, :, :, :])
                fwm_sb = pfw.tile([9, C], BF16, name="fwm_sb")
                nc.sync.dma_start(out=fwm_sb, in_=fwm_d[:, :])
                im2 = pfw.tile([9, PITCH * PITCH], BF16, name="im2")
                nc.gpsimd.memset(im2, 0.0)
                for dy in range(3):
                    for dx in range(3):
                        j = dy * 3 + dx
                        joff = dy * PITCH + dx
                        nc.sync.dma_start(
                            out=im2[j:j + 1, 0:PITCH * PITCH - joff],
                            in_=mask_dram[:, joff:])
                ehl = {}
                with tc.tile_pool(name="ps3", bufs=1, space="PSUM") as ps3, \
                     tc.tile_pool(name="ps3n", bufs=1, space="PSUM") as ps3n, \
                     tc.tile_pool(name="ev3", bufs=2) as ev, \
                     tc.tile_pool(name="nump", bufs=1) as num_p, \
                     tc.tile_pool(name="xbp", bufs=2) as xb_p, \
                     tc.tile_pool(name="rbp", bufs=2) as rb_p:
                    xv = x_d.rearrange("(t p) (h q) -> t p h q", p=128, q=PITCH)
                    for di in (1, 0):
                        stats_dram = dpool.tile([2, N], F32, name=f"stats_dram{di}",
                                                tag="stats_dram", bufs=2)
                        rmur_dram = dpool.tile([2, N], BF16, name=f"rmur_dram{di}",
                                               tag="rmur_dram", bufs=2)
                        k_bf, v_bf, vc_bf = kvs[di]
                        num_sb = [num_p.tile([128, N], BF16, name=f"num{di}{cb_i}",
                                             tag=f"num{cb_i}") for cb_i in range(2)]

                        for bi, (y0, nr) in enumerate(BLOCKS):
                            nn = nr * W
                            qps = ps3n.tile([32, nn], F32, name="qps", tag="qps")
                            qrhs = (msk8[:, :, y0:y0 + nr, 0:W] if di == 1
                                    else x8r[:, :, y0:y0 + nr, 1:89])
                            nc.tensor.matmul(qps,
                                             kq8_sb[:, :, di * 32:di * 32 + 32],
                                             qrhs, start=True, stop=True,
                                             perf_mode=DR)
                            q_bf = ev.tile([32, nn], BF16, name="q_bf", tag="q_bf")
                            nc.scalar.activation(
                                q_bf, qps, AF.Identity,
                                bias=cb_sb[0:32, CB_QB0 + 2 * di:CB_QB0 + 2 * di + 1],
                                scale=1.0 / SWX)
                            sps = ps3.tile([M2, nn], F32, name="sps", tag="sps")
                            nc.tensor.matmul(sps, k_bf, q_bf, start=True, stop=True)
                            e_bf = ev.tile([M2, nn], BF16, name="e_bf", tag="e_bf")
                            nc.scalar.activation(e_bf, sps, AF.Exp)
                            mu_ps = ps3n.tile([1, nn], F32, name="amups", tag="astps",
                                              bufs=2)
                            nc.tensor.matmul(mu_ps, vc_bf, e_bf, start=True, stop=True)
                            sq_ps = ps3n.tile([1, nn], F32, name="asqps", tag="astps",
                                              bufs=2)
                            nsq8 = ev.tile([128, 2, nn], F8, name="nsq8", tag="nsq8")
                            for cb_i in range(2):
                                nps = ps3.tile([128, nn], F32, name="nps",
                                               tag=f"nps{cb_i}", bufs=2)
                                nc.tensor.matmul(nps,
                                                 v_bf[:, cb_i * 128:(cb_i + 1) * 128],
                                                 e_bf, start=True, stop=True)
                                nc.vector.tensor_copy(
                                    num_sb[cb_i][:, y0 * W:y0 * W + nn], nps)
                                nc.scalar.activation(nsq8[:, cb_i, :], nps, AF.Square,
                                                     scale=SQS)
                            nc.tensor.matmul(sq_ps, ones8, nsq8, start=True,
                                             stop=True, perf_mode=DR)
                            mrow = ev.tile([1, nn], F32, name="mrow", tag="mrow")
                            nc.vector.tensor_scalar(mrow, mu_ps, 1.0 / C, None,
                                                    ALU.mult)
                            nc.sync.dma_start(out=stats_dram[0:1, y0 * W:y0 * W + nn],
                                              in_=mrow)
                            srow = ev.tile([1, nn], F32, name="srow", tag="srow")
                            nc.scalar.activation(srow, sq_ps, AF.Identity,
                                                 scale=1.0 / (SQS * SQS * C))
                            nc.sync.dma_start(out=stats_dram[1:2, y0 * W:y0 * W + nn],
                                              in_=srow)

                        if dbg:
                            for cb_i in range(2):
                                nc.sync.dma_start(
                                    out=dbg_d[f"num{di}"][cb_i * 128:(cb_i + 1) * 128, :],
                                    in_=num_sb[cb_i])
                        # this dir's enh hi/lo fp8 target (scrA/scrB now dead)
                        hi_t = scr.tile([128, 2, H, PITCH], F8, name=f"ehi{di}",
                                        tag=("scrA" if di == 0 else "scrB"))
                        lo_t = scr.tile([128, 2, H, PITCH], F8, name=f"elo{di}",
                                        tag=("scrC" if di == 0 else "scrD"))
                        nc.vector.memset(hi_t[:, :, :, 0::89], 0.0)
                        nc.vector.memset(lo_t[:, :, :, 0::89], 0.0)
                        ehl[di] = (hi_t, lo_t)

                        for ch in range(NCH):
                            c0 = ch * CHUNK_ROWS * W
                            cn = CHUNK_ROWS * W
                            rows = slice(ch * CHUNK_ROWS, (ch + 1) * CHUNK_ROWS)
                            mu_t = ev.tile([M2, STJ], F32, name="mu_t", tag="mu_t")
                            nc.sync.dma_start(
                                out=mu_t,
                                in_=stats_dram[0, c0:c0 + cn]
                                .rearrange("(p j) -> p j", j=STJ))
                            ms_t = ev.tile([M2, STJ], F32, name="ms_t", tag="ms_t")
                            nc.sync.dma_start(
                                out=ms_t,
                                in_=stats_dram[1, c0:c0 + cn]
                                .rearrange("(p j) -> p j", j=STJ))
                            mu2_t = ev.tile([M2, STJ], F32, name="mu2_t", tag="mu2_t")
                            nc.vector.tensor_tensor(out=mu2_t, in0=mu_t, in1=mu_t,
                                                    op=ALU.mult)
                            nc.vector.tensor_tensor(out=ms_t, in0=ms_t, in1=mu2_t,
                                                    op=ALU.subtract)
                            sd_t = ev.tile([M2, STJ], F32, name="sd_t", tag="sd_t")
                            nc.scalar.activation(sd_t, ms_t, AF.Sqrt,
                                                 bias=eps_sb[0:M2, :], scale=1.0)
                            r_t = ev.tile([M2, STJ], F32, name="r_t", tag="r_t")
                            nc.vector.reciprocal(r_t, sd_t)
                            rm_bf = ev.tile([M2, 2, STJ], BF16, name="rm_bf",
                                            tag="rm_bf")
                            nc.vector.tensor_copy(rm_bf[:, 0, :], r_t)
                            nc.vector.tensor_tensor(out=mu_t, in0=mu_t, in1=r_t,
                                                    op=ALU.mult)
                            nc.vector.tensor_copy(rm_bf[:, 1, :], mu_t)
                            nc.sync.dma_start(
                                out=rmur_dram[:, c0:c0 + cn]
                                .rearrange("t (p j) -> p t j", j=STJ), in_=rm_bf)
                            r_b = rb_p.tile([128, cn], BF16, name="r_b", tag="r_b")
                            nc.sync.dma_start(
                                out=r_b,
                                in_=rmur_dram[0:1, c0:c0 + cn].to_broadcast([128, cn]))
                            mur_b = rb_p.tile([128, cn], BF16, name="mur_b",
                                              tag="mur_b")
                            nc.sync.dma_start(
                                out=mur_b,
                                in_=rmur_dram[1:2, c0:c0 + cn].to_broadcast([128, cn]))
                            for cb_i in range(2):
                                xb_t = xb_p.tile([128, CHUNK_ROWS, PITCH], BF16,
                                                 name="xb_t", tag=f"xb{cb_i}")
                                nc.sync.dma_start(out=xb_t,
                                                  in_=xv[2 * di + cb_i][:, rows, :])
                                seg = ev.tile([128, cn], BF16, name="seg",
                                              tag=f"seg{cb_i}")
                                nc.vector.tensor_tensor(
                                    out=seg, in0=num_sb[cb_i][:, c0:c0 + cn],
                                    in1=r_b, op=ALU.mult)
                                nc.vector.tensor_tensor(out=seg, in0=seg, in1=mur_b,
                                                        op=ALU.subtract)
                                nc.scalar.activation(seg, seg, AF.Identity,
                                                     bias=cbc(CB_NB0 + 2 * di, cb_i),
                                                     scale=cbc(CB_NG0 + 2 * di, cb_i))
                                segr = seg.rearrange("p (h w) -> p h w", w=W)
                                nc.vector.tensor_tensor(
                                    out=segr, in0=segr,
                                    in1=xb_t[:, :, 1:89], op=ALU.add)
                                nc.scalar.activation(hi_t[:, cb_i, rows, 1:89], segr,
                                                     AF.Identity)
                                nc.vector.tensor_tensor(
                                    out=lo_t[:, cb_i, rows, 1:89], in0=segr,
                                    in1=hi_t[:, cb_i, rows, 1:89], op=ALU.subtract)

                if dbg:
                    for di in range(2):
                        hi_t, lo_t = ehl[di]
                        for cb_i in range(2):
                            pl = 2 * di + cb_i
                            tmp = pfw.tile([128, H * PITCH], BF16, name=f"dbgt{pl}")
                            nc.vector.tensor_copy(
                                tmp, hi_t[:, cb_i].rearrange("p h q -> p (h q)"))
                            nc.sync.dma_start(
                                out=dbg_d["ehi"][:, pl * H * PITCH:(pl + 1) * H * PITCH],
                                in_=tmp)
                            nc.vector.tensor_copy(
                                tmp, lo_t[:, cb_i].rearrange("p h q -> p (h q)"))
                            nc.sync.dma_start(
                                out=dbg_d["elo"][:, pl * H * PITCH:(pl + 1) * H * PITCH],
                                in_=tmp)

                # ================= Phase 4: conv2 =================
                if maxphase < 4:
                    return
                with tc.tile_pool(name="ps4", bufs=4, space="PSUM") as ps4, \
                     tc.tile_pool(name="ev4", bufs=2) as ev:
                    im2v = im2.rearrange("o (h q) -> o h q", q=PITCH)
                    hi_r, lo_r = ehl[0]
                    hi_d, lo_d = ehl[1]
                    # slots: (term, pair): term 0 = Wh*Xh, 1 = Wh*Xl, 2 = Wl*Xh
                    slot_w = [fwh_sb, fwh_sb, fwl_sb]
                    slot_x = [(hi_r, hi_d), (lo_r, lo_d), (hi_r, hi_d)]

                    def rhs2(sl, rlo, rhi, dx):
                        return slot_x[sl // 2][sl % 2][:, :, rlo:rhi, dx:dx + W]

                    for y0, nr in BLOCKS:
                        nn = nr * W
                        for cb_i in range(2):
                            ps = ps4.tile([128, nr, W], F32, name="c2ps", tag="c2ps")
                            psf = ps.rearrange("p r w -> p (r w)")
                            _conv3x3_dr(nc, psf,
                                        lambda o, sl, cb_i=cb_i:
                                            slot_w[sl // 2]
                                            [:, sl % 2, :, o,
                                             cb_i * 128:(cb_i + 1) * 128],
                                        rhs2, y0, nr, 6, stop_last=False)
                            nc.tensor.matmul(
                                psf, fwm_sb[:, cb_i * 128:(cb_i + 1) * 128],
                                im2v[:, y0:y0 + nr, 0:W], start=False, stop=True)
                            o_t = ev.tile([128, nn], F32, name="o_t", tag="o_t")
                            nc.scalar.activation(o_t, psf, AF.Relu,
                                                 bias=cbc(CB_FT, cb_i),
                                                 scale=cbc(CB_FS, cb_i))
                            nc.sync.dma_start(
                                out=out_d[cb_i * 128:(cb_i + 1) * 128,
                                          y0 * W:y0 * W + nn],
                                in_=o_t)
    nc.finalize()
    return nc


def kernel(**inputs):
    in_maps, b2 = _prep(inputs)
    key = ("nc", round(b2, 9))
    if key not in _CACHE:
        nc = bacc.Bacc("TRN2", target_bir_lowering=False, debug=False)
        _build(nc, b2)
        _CACHE[key] = nc
    nc = _CACHE[key]
    res = run_bass_kernel_spmd(nc, in_maps, list(range(B)))
    return np.stack([np.asarray(res.results[i]["out"], np.float32).reshape(C, H, W)
                     for i in range(B)])
